# revision 1
# baseline (speedup 1.0000x reference)
"""TRN2 Bass kernel for nn_Aggregator (GNN message passing aggregator).

Strategy (8 NeuronCores, SPMD):
  - Sort edges by head (host), shard by head range: core c owns entities
    [c*12544, (c+1)*12544) and all edges whose head falls in that range.
  - Per 128-entity tile, edges are padded to CPT chunks of 128 slots;
    compute is batched per tile (S = CPT*128 edge slots) in bf16 with
    f32 PSUM accumulation.
  - Stage A: per-edge gather of entity_emb[tail] (indirect DMA, 128
    rows/call, Q7 descriptor-gen bound), one-hot matmul machinery computes
    attention + unnormalized kg per tile in PSUM (factor-out normalization
    -> single pass), then G = (kg^2) @ (weight^2).T per tile ([ent, 32]).
  - AllGather G (the only collective: the edge weight
    w = (|kg_h*rel| |kg_t*rel|)^2 equals G[h,type]*G[t,type]).
  - Stage B: per-edge gather of combined bf16 rows [G(32) | 1 | emb(128)]
    built in DRAM; unstable scatter-softmax (exact here: max w ~ 6e-4);
    one fused matmul accumulates [s | out_unnorm] per tile; output = shard.
"""
import sys

for _p in ("/opt/trn_rl_repo", "/root/.axon_site/_ro/trn_rl_repo"):
    if _p not in sys.path:
        sys.path.insert(0, _p)

import numpy as np
import ml_dtypes

import concourse.bass as bass
import concourse.bacc as bacc
import concourse.mybir as mybir
import concourse.tile as tile
from concourse.bass import IndirectOffsetOnAxis
from concourse.bass_utils import run_bass_kernel_spmd
from concourse.masks import make_identity

FP = mybir.dt.float32
BF = mybir.dt.bfloat16
I32 = mybir.dt.int32

# Problem constants
N_ENT = 100000
D = 128
H = 4
DH = 32
R = 32
NCORE = 8
TILE = 128
TPC = 98            # tiles per core
N_PER = TPC * TILE  # 12544
N_PAD = N_PER * NCORE  # 100352
CPT = 5             # chunks (128 edge slots) per tile; max tile load is 576
CW = 161            # combined row: 32 G | 1 ones | 128 emb


def _bcast(src_ap, parts):
    """Partition-broadcast a [1, S] DRAM AP to [parts, S] for DMA."""
    return bass.AP(tensor=src_ap.tensor, offset=src_ap.offset,
                   ap=[[0, parts]] + [list(p) for p in src_ap.ap[1:]])


def build(ncore=NCORE, tpc=TPC, cpt=CPT, n_tab=N_ENT, with_cc=True):
    n_per = tpc * TILE
    n_pad = n_per * ncore
    nch = tpc * cpt
    S = cpt * 128

    nc = bacc.Bacc()
    ent = nc.dram_tensor("ent", [n_pad, D], BF, kind="ExternalInput")
    myrowsT = nc.dram_tensor("myrowsT", [n_per, D], BF, kind="ExternalInput")
    tailidx = nc.dram_tensor("tailidx", [128, nch], I32, kind="ExternalInput")
    hloc = nc.dram_tensor("hloc", [128, nch], FP, kind="ExternalInput")
    typ = nc.dram_tensor("typ", [128, nch], FP, kind="ExternalInput")
    hrow = nc.dram_tensor("hrow", [tpc, S], FP, kind="ExternalInput")
    trow = nc.dram_tensor("trow", [tpc, S], FP, kind="ExternalInput")
    qT = nc.dram_tensor("qT", [D, D], BF, kind="ExternalInput")
    kT = nc.dram_tensor("kT", [D, D], BF, kind="ExternalInput")
    vT = nc.dram_tensor("vT", [D, D], BF, kind="ExternalInput")
    wgt = nc.dram_tensor("wgt", [R, D], BF, kind="ExternalInput")
    w2T = nc.dram_tensor("w2T", [D, R], BF, kind="ExternalInput")
    hmask = nc.dram_tensor("hmask", [D, H], BF, kind="ExternalInput")
    hmaskT = nc.dram_tensor("hmaskT", [H, D], BF, kind="ExternalInput")
    iota_in = nc.dram_tensor("iota", [128, 128], FP, kind="ExternalInput")
    iotac_in = nc.dram_tensor("iotac", [128, S], FP, kind="ExternalInput")
    out_d = nc.dram_tensor("out", [n_per, D], FP, kind="ExternalOutput")

    g_my = nc.dram_tensor("g_my", [n_per, R + 1], BF)
    if ncore > 4:
        g_full = nc.dram_tensor("g_full", [n_pad, R + 1], BF, addr_space="Shared")
    else:
        g_full = nc.dram_tensor("g_full", [n_pad, R + 1], BF)
    comb = nc.dram_tensor("comb", [n_pad, CW], BF)

    def mmN(out_fn, lhsT, rhs_fn, n_total, **kw):
        for off in range(0, n_total, 512):
            n = min(512, n_total - off)
            nc.tensor.matmul(out=out_fn(off, n), lhsT=lhsT, rhs=rhs_fn(off, n),
                             start=True, stop=True, **kw)

    with tile.TileContext(nc) as tc:
        with (
            tc.tile_pool(name="consts", bufs=1) as consts,
            tc.tile_pool(name="asb", bufs=2) as asb,
            tc.tile_pool(name="psA", bufs=2, space="PSUM") as psA,
            tc.tile_pool(name="psB", bufs=2, space="PSUM") as psB,
            tc.tile_pool(name="acc", bufs=1, space="PSUM") as accp,
            tc.tile_pool(name="tsb", bufs=2) as tsb,
        ):
            # ---------- constants / indices ----------
            ident = consts.tile([128, 128], BF, tag="ident")
            make_identity(nc, ident[:])
            qT_s = consts.tile([D, D], BF, tag="qT")
            kT_s = consts.tile([D, D], BF, tag="kT")
            vT_s = consts.tile([D, D], BF, tag="vT")
            wgt_s = consts.tile([R, D], BF, tag="wgt")
            w2T_s = consts.tile([D, R], BF, tag="w2T")
            hm_s = consts.tile([D, H], BF, tag="hm")
            hmT_s = consts.tile([H, D], BF, tag="hmT")
            iota_s = consts.tile([128, 128], FP, tag="iota")
            iotac_s = consts.tile([128, S], FP, tag="iotac")
            for dst, src in ((qT_s, qT), (kT_s, kT), (vT_s, vT), (wgt_s, wgt),
                             (w2T_s, w2T), (hm_s, hmask), (hmT_s, hmaskT),
                             (iota_s, iota_in), (iotac_s, iotac_in)):
                nc.sync.dma_start(out=dst[:], in_=src[:])
            tidx_s = consts.tile([128, nch], I32, tag="tidx")
            hloc_s = consts.tile([128, nch], FP, tag="hloc")
            typ_s = consts.tile([128, nch], FP, tag="typ")
            nc.sync.dma_start(out=tidx_s[:], in_=tailidx[:])
            nc.sync.dma_start(out=hloc_s[:], in_=hloc[:])
            nc.sync.dma_start(out=typ_s[:], in_=typ[:])

            # combined-table emb part (overlaps stage A); 16-bit AP counts
            hh = n_pad // 2
            nc.sync.dma_start(out=comb[0:hh, R + 1:CW], in_=ent[0:hh, :])
            nc.sync.dma_start(out=comb[hh:n_pad, R + 1:CW], in_=ent[hh:n_pad, :])

            # ---------- stage A ----------
            for t in range(tpc):
                E_T = tsb.tile([128, D], BF, tag="E_T")
                nc.sync.dma_start(out=E_T[:], in_=myrowsT[t * 128:(t + 1) * 128, :])
                q_ps = psB.tile([128, 512], FP, tag="B")
                nc.tensor.matmul(out=q_ps[:, 0:128], lhsT=E_T[:], rhs=qT_s[:],
                                 start=True, stop=True)
                Q_s = tsb.tile([128, D], BF, tag="Q_s")
                nc.vector.tensor_copy(out=Q_s[:], in_=q_ps[:, 0:128])

                # broadcast head/type rows for this tile
                hbc = tsb.tile([128, S], FP, tag="hbc")
                nc.sync.dma_start(out=hbc[:], in_=_bcast(hrow[t:t + 1, :], 128))
                tbc = tsb.tile([R, S], FP, tag="tbc")
                nc.sync.dma_start(out=tbc[:], in_=_bcast(trow[t:t + 1, :], R))

                # gathered tails for the whole tile
                Te = asb.tile([128, cpt, D], BF, tag="Te")
                for k in range(cpt):
                    nc.gpsimd.indirect_dma_start(
                        out=Te[:, k, :], out_offset=None, in_=ent[:],
                        in_offset=IndirectOffsetOnAxis(
                            ap=tidx_s[:, t * cpt + k:t * cpt + k + 1], axis=0),
                    )
                # one-hots: edge-major oh_e/oR_e (columns), ent-major oh_ent,
                # feature-major oRT (rows, via broadcast + iota compare)
                oh_e = asb.tile([128, cpt, 128], BF, tag="oh_e")
                oR_e = asb.tile([128, cpt, R], BF, tag="oR_e")
                for k in range(cpt):
                    j = t * cpt + k
                    nc.vector.tensor_tensor(
                        out=oh_e[:, k, :],
                        in0=hloc_s[:, j:j + 1].to_broadcast([128, 128]),
                        in1=iota_s[:], op=mybir.AluOpType.is_equal)
                    nc.vector.tensor_tensor(
                        out=oR_e[:, k, :],
                        in0=typ_s[:, j:j + 1].to_broadcast([128, R]),
                        in1=iota_s[:, 0:R], op=mybir.AluOpType.is_equal)
                oh_ent = asb.tile([128, S], BF, tag="oh_ent")
                nc.vector.tensor_tensor(out=oh_ent[:], in0=hbc[:], in1=iotac_s[:],
                                        op=mybir.AluOpType.is_equal)
                oRT = asb.tile([R, S], BF, tag="oRT")
                nc.vector.tensor_tensor(out=oRT[:], in0=tbc[:],
                                        in1=iotac_s[0:R, :],
                                        op=mybir.AluOpType.is_equal)

                # T_T via PE transposes
                tt_ps = psA.tile([128, S], BF, tag="A")
                for k in range(cpt):
                    nc.tensor.transpose(out=tt_ps[:, k * 128:(k + 1) * 128],
                                        in_=Te[:, k, :], identity=ident[:])
                T_T = asb.tile([128, S], BF, tag="T_T")
                nc.vector.tensor_copy(out=T_T[:], in_=tt_ps[:])

                # rel_T = wgt.T @ oRT; neigh_T = T_T * rel_T
                rel_ps = psA.tile([128, S], FP, tag="A")
                mmN(lambda o, n: rel_ps[:, o:o + n], wgt_s[:],
                    lambda o, n: oRT[:, o:o + n], S)
                neigh = asb.tile([128, S], BF, tag="neigh")
                nc.vector.tensor_mul(out=neigh[:], in0=T_T[:], in1=rel_ps[:])
                # k_T = kT.T @ neigh
                k_ps = psA.tile([128, S], FP, tag="A")
                mmN(lambda o, n: k_ps[:, o:o + n], kT_s[:],
                    lambda o, n: neigh[:, o:o + n], S)
                kTs = asb.tile([128, S], BF, tag="kTs")
                nc.scalar.activation(out=kTs[:], in_=k_ps[:],
                                     func=mybir.ActivationFunctionType.Copy)
                # q_edge_T = Q.T @ oh_ent
                qe_ps = psA.tile([128, S], FP, tag="A")
                mmN(lambda o, n: qe_ps[:, o:o + n], Q_s[:],
                    lambda o, n: oh_ent[:, o:o + n], S)
                qk = asb.tile([128, S], BF, tag="qk")
                nc.vector.tensor_mul(out=qk[:], in0=kTs[:], in1=qe_ps[:])
                # att = hmask.T @ qk  [H, S] (reuse qe_ps rows 0:H after qk read)
                mmN(lambda o, n: qe_ps[0:H, o:o + n], hm_s[:],
                    lambda o, n: qk[:, o:o + n], S)
                attc = asb.tile([H, S], FP, tag="attc")
                nc.vector.tensor_scalar_min(out=attc[:], in0=qe_ps[0:H, 0:S],
                                            scalar1=10.0)
                nc.vector.tensor_scalar_max(out=attc[:], in0=attc[:], scalar1=-10.0)
                expT = asb.tile([H, S], BF, tag="expT")
                nc.scalar.activation(out=expT[:], in_=attc[:],
                                     func=mybir.ActivationFunctionType.Exp)
                # v_e, att_exp_e, expatt_e per chunk
                v_ps = psA.tile([128, S], FP, tag="A")
                ae_ps = psA.tile([128, S], FP, tag="A")
                eeB = psB.tile([128, 512], BF, tag="B")
                ee_ps = eeB[:, 0:cpt * 4]
                for k in range(cpt):
                    sl = slice(k * 128, (k + 1) * 128)
                    nc.tensor.matmul(out=v_ps[:, sl], lhsT=neigh[:, sl],
                                     rhs=vT_s[:], start=True, stop=True)
                    nc.tensor.matmul(out=ae_ps[:, sl], lhsT=expT[:, sl],
                                     rhs=hmT_s[:], start=True, stop=True)
                    nc.tensor.transpose(out=ee_ps[:, k * 4:(k + 1) * 4] if False else eeB[:, k * 4:(k + 1) * 4],
                                        in_=expT[:, sl], identity=ident[0:H, 0:H])
                v_sb = asb.tile([128, S], BF, tag="v_sb")
                nc.vector.tensor_copy(out=v_sb[:], in_=v_ps[:])
                vx = asb.tile([128, cpt, 132], BF, tag="vx")
                nc.vector.tensor_mul(
                    out=vx[:, :, 0:128],
                    in0=v_sb[:].rearrange("p (c e) -> p c e", c=cpt),
                    in1=ae_ps[:].rearrange("p (c e) -> p c e", c=cpt))
                nc.vector.tensor_copy(
                    out=vx[:, :, 128:132],
                    in_=ee_ps.rearrange("p (c e) -> p c e", c=cpt))
                kgu = accp.tile([128, 132], FP, tag="kgu")
                for k in range(cpt):
                    nc.tensor.matmul(out=kgu[:, 0:132], lhsT=oh_e[:, k, :],
                                     rhs=vx[:, k, :],
                                     start=(k == 0), stop=(k == cpt - 1))

                # tile epilogue: kg, G
                rnorm = tsb.tile([128, H], FP, tag="rnorm")
                nc.vector.tensor_scalar_add(out=rnorm[:], in0=kgu[:, 128:132],
                                            scalar1=1e-8)
                nc.vector.reciprocal(out=rnorm[:], in_=rnorm[:])
                kg_sb = tsb.tile([128, D], BF, tag="kg_sb")
                for h in range(H):
                    nc.vector.tensor_scalar_mul(
                        out=kg_sb[:, h * DH:(h + 1) * DH],
                        in0=kgu[:, h * DH:(h + 1) * DH],
                        scalar1=rnorm[:, h:h + 1])
                gp = psB.tile([128, 512], BF, tag="B")
                nc.tensor.transpose(out=gp[:, 0:128], in_=kg_sb[:], identity=ident[:])
                kg2T = tsb.tile([128, 128], BF, tag="kg2T")
                nc.scalar.square(out=kg2T[:], in_=gp[:, 0:128])
                gf = psB.tile([128, 512], FP, tag="B")
                nc.tensor.matmul(out=gf[:, 0:R], lhsT=kg2T[:], rhs=w2T_s[:],
                                 start=True, stop=True)
                g_sb = tsb.tile([128, R + 1], BF, tag="g_sb")
                nc.vector.tensor_copy(out=g_sb[:, 0:R], in_=gf[:, 0:R])
                nc.vector.memset(g_sb[:, R:R + 1], 1.0)
                nc.sync.dma_start(out=g_my[t * 128:(t + 1) * 128, :], in_=g_sb[:])

            # ---------- AllGather G ----------
            if with_cc:
                nc.gpsimd.collective_compute(
                    "AllGather", mybir.AluOpType.bypass,
                    replica_groups=[list(range(ncore))],
                    ins=[g_my[:, :]], outs=[g_full[:, :]],
                )
                nc.sync.dma_start(out=comb[0:hh, 0:R + 1], in_=g_full[0:hh, :])
                nc.sync.dma_start(out=comb[hh:n_pad, 0:R + 1], in_=g_full[hh:n_pad, :])
            else:
                nc.sync.dma_start(out=g_full[0:n_per, :], in_=g_my[:, :])
                nc.sync.dma_start(out=comb[0:n_per, 0:R + 1], in_=g_my[:, :])

            # ---------- stage B ----------
            for t in range(tpc):
                g_tile = tsb.tile([128, R], BF, tag="g_tile")
                nc.sync.dma_start(out=g_tile[:], in_=g_my[t * 128:(t + 1) * 128, 0:R])
                gt_ps = psB.tile([128, 512], BF, tag="B")
                nc.tensor.transpose(out=gt_ps[0:R, 0:128], in_=g_tile[:],
                                    identity=ident[:])
                G_T = tsb.tile([R, 128], BF, tag="G_T")
                nc.vector.tensor_copy(out=G_T[:], in_=gt_ps[0:R, 0:128])
                tbc = tsb.tile([R, S], FP, tag="tbc")
                nc.sync.dma_start(out=tbc[:], in_=_bcast(trow[t:t + 1, :], R))

                Ce = asb.tile([128, cpt, CW], BF, tag="Ce")
                for k in range(cpt):
                    nc.gpsimd.indirect_dma_start(
                        out=Ce[:, k, :], out_offset=None, in_=comb[:],
                        in_offset=IndirectOffsetOnAxis(
                            ap=tidx_s[:, t * cpt + k:t * cpt + k + 1], axis=0),
                    )
                oh_e = asb.tile([128, cpt, 128], BF, tag="oh_e")
                oR_e = asb.tile([128, cpt, R], BF, tag="oR_e")
                for k in range(cpt):
                    j = t * cpt + k
                    nc.vector.tensor_tensor(
                        out=oh_e[:, k, :],
                        in0=hloc_s[:, j:j + 1].to_broadcast([128, 128]),
                        in1=iota_s[:], op=mybir.AluOpType.is_equal)
                    nc.vector.tensor_tensor(
                        out=oR_e[:, k, :],
                        in0=typ_s[:, j:j + 1].to_broadcast([128, R]),
                        in1=iota_s[:, 0:R], op=mybir.AluOpType.is_equal)
                oRT = asb.tile([R, S], BF, tag="oRT")
                nc.vector.tensor_tensor(out=oRT[:], in0=tbc[:],
                                        in1=iotac_s[0:R, :],
                                        op=mybir.AluOpType.is_equal)
                # M1_T [e, ent] per chunk; hr2/tr2 via mult+reduce
                m1_ps = psA.tile([128, S], FP, tag="A")
                for k in range(cpt):
                    nc.tensor.matmul(out=m1_ps[:, k * 128:(k + 1) * 128],
                                     lhsT=oRT[:, k * 128:(k + 1) * 128],
                                     rhs=G_T[:], start=True, stop=True)
                scr = asb.tile([128, S], BF, tag="scr")
                nc.vector.tensor_mul(
                    out=scr[:],
                    in0=m1_ps[:],
                    in1=oh_e[:].rearrange("p c e -> p (c e)"))
                hr2 = asb.tile([128, cpt], FP, tag="hr2")
                nc.vector.tensor_reduce(
                    out=hr2[:], in_=scr[:].rearrange("p (c e) -> p c e", c=cpt),
                    axis=mybir.AxisListType.X, op=mybir.AluOpType.add)
                scr2 = asb.tile([128, cpt * R], FP, tag="scr2")
                nc.vector.tensor_mul(
                    out=scr2[:].rearrange("p (c r) -> p c r", c=cpt),
                    in0=Ce[:, :, 0:R],
                    in1=oR_e[:])
                tr2 = asb.tile([128, cpt], FP, tag="tr2")
                nc.vector.tensor_reduce(
                    out=tr2[:], in_=scr2[:].rearrange("p (c r) -> p c r", c=cpt),
                    axis=mybir.AxisListType.X, op=mybir.AluOpType.add)
                expw = asb.tile([128, cpt], FP, tag="expw")
                nc.vector.tensor_mul(out=expw[:], in0=hr2[:], in1=tr2[:])
                nc.scalar.activation(out=expw[:], in_=expw[:],
                                     func=mybir.ActivationFunctionType.Exp)
                mske = asb.tile([128, cpt, 128], BF, tag="mske")
                sout = accp.tile([128, 132], FP, tag="kgu")
                for k in range(cpt):
                    nc.vector.tensor_scalar_mul(out=mske[:, k, :],
                                                in0=oh_e[:, k, :],
                                                scalar1=expw[:, k:k + 1])
                    nc.tensor.matmul(out=sout[:, 0:129], lhsT=mske[:, k, :],
                                     rhs=Ce[:, k, R:CW],
                                     start=(k == 0), stop=(k == cpt - 1))

                rs = tsb.tile([128, 1], FP, tag="rs")
                nc.vector.tensor_scalar_add(out=rs[:], in0=sout[:, 0:1],
                                            scalar1=1e-30)
                nc.vector.reciprocal(out=rs[:], in_=rs[:])
                o_sb = tsb.tile([128, D], FP, tag="o_sb")
                nc.vector.tensor_scalar_mul(out=o_sb[:], in0=sout[:, 1:129],
                                            scalar1=rs[:])
                nc.sync.dma_start(out=out_d[t * 128:(t + 1) * 128, :], in_=o_sb[:])

    nc.finalize()
    return nc


def host_prep(entity_emb, weight, qTrans, kTrans, vTrans, edge_index, edge_type,
              ncore=NCORE, tpc=TPC, cpt=CPT, n_tab=N_ENT):
    """Sort/shard/pad edges; build all per-core input dicts."""
    n_per = tpc * TILE
    nch = tpc * cpt
    slots = cpt * 128

    head = np.asarray(edge_index[0], dtype=np.int64)
    tail = np.asarray(edge_index[1], dtype=np.int64)
    etype = np.asarray(edge_type, dtype=np.int64) - 1

    order = np.argsort(head, kind="stable")
    hs, ts, rs = head[order], tail[order], etype[order]
    tile_of = hs // TILE
    n_tiles = ncore * tpc
    counts = np.bincount(tile_of, minlength=n_tiles)
    assert counts.max() <= slots, f"tile overflow: {counts.max()} > {slots}"
    tstart = np.concatenate([[0], np.cumsum(counts)])

    tails_sl = np.zeros((ncore, tpc, slots), dtype=np.int32)
    hloc_sl = np.full((ncore, tpc, slots), 255, dtype=np.float32)
    type_sl = np.full((ncore, tpc, slots), R, dtype=np.float32)
    for g in range(n_tiles):
        c, t = g // tpc, g % tpc
        n = counts[g]
        sl = slice(tstart[g], tstart[g] + n)
        tails_sl[c, t, :n] = ts[sl]
        hloc_sl[c, t, :n] = hs[sl] - g * TILE
        type_sl[c, t, :n] = rs[sl]

    def to_dev(a, dt):
        return np.ascontiguousarray(
            a.reshape(ncore, nch, 128).transpose(0, 2, 1)).astype(dt)

    tails_d = to_dev(tails_sl, np.int32)
    hloc_d = to_dev(hloc_sl, np.float32)
    type_d = to_dev(type_sl, np.float32)

    n_pad_rows = ncore * n_per
    ent_raw = np.asarray(entity_emb, dtype=np.float32)
    ent = np.zeros((n_pad_rows, D), np.float32)
    ent[:ent_raw.shape[0]] = ent_raw
    ent_bf = ent.astype(ml_dtypes.bfloat16)

    wgt = np.asarray(weight, dtype=np.float32)
    w2T = np.ascontiguousarray((wgt ** 2).T)
    hmask = np.zeros((D, H), np.float32)
    for h in range(H):
        hmask[h * DH:(h + 1) * DH, h] = 1.0
    hmaskT = np.ascontiguousarray(hmask.T)
    iota = np.tile(np.arange(128, dtype=np.float32), (128, 1))
    iotac = np.tile(np.arange(128, dtype=np.float32)[:, None], (1, slots))

    shared = {
        "ent": ent_bf,
        "qT": np.asarray(qTrans, np.float32).astype(ml_dtypes.bfloat16),
        "kT": np.asarray(kTrans, np.float32).astype(ml_dtypes.bfloat16),
        "vT": np.asarray(vTrans, np.float32).astype(ml_dtypes.bfloat16),
        "wgt": wgt.astype(ml_dtypes.bfloat16),
        "w2T": w2T.astype(ml_dtypes.bfloat16),
        "hmask": hmask.astype(ml_dtypes.bfloat16),
        "hmaskT": hmaskT.astype(ml_dtypes.bfloat16),
        "iota": iota, "iotac": iotac,
    }
    in_maps = []
    for c in range(ncore):
        rows = ent[c * n_per:(c + 1) * n_per]
        myT = rows.reshape(tpc, TILE, D).transpose(0, 2, 1).reshape(n_per, D)
        in_maps.append(dict(
            shared,
            myrowsT=np.ascontiguousarray(myT).astype(ml_dtypes.bfloat16),
            tailidx=tails_d[c],
            hloc=hloc_d[c], typ=type_d[c],
            hrow=np.ascontiguousarray(hloc_sl[c]).astype(np.float32),
            trow=np.ascontiguousarray(type_sl[c]).astype(np.float32),
        ))
    return in_maps


_NC_CACHE = {}


def kernel(entity_emb, user_emb, interact_mat, weight, qTrans, kTrans, vTrans,
           edge_index, edge_type, layer=0):
    key = "full"
    if key not in _NC_CACHE:
        _NC_CACHE[key] = build()
    nc = _NC_CACHE[key]
    in_maps = host_prep(entity_emb, weight, qTrans, kTrans, vTrans,
                        edge_index, edge_type)
    res = run_bass_kernel_spmd(nc, in_maps, list(range(NCORE)))
    out = np.concatenate([res.results[c]["out"] for c in range(NCORE)], axis=0)
    return np.ascontiguousarray(out[:N_ENT]).astype(np.float32)



# revision 25
# speedup vs baseline: 1.5495x; 1.5495x over previous
"""TRN2 Bass kernel for nn_Aggregator (GNN message passing aggregator).

Strategy (8 NeuronCores, SPMD):
  - Sort edges by head (host), shard by head range: core c owns entities
    [c*12544, (c+1)*12544) and all edges whose head falls in that range.
  - Stage A: per-edge gather of entity_emb[tail] (indirect DMA, 128
    rows/call) into SBUF-resident per-tile tiles (kept for stage B);
    feature-major qkv via one-hot matmuls; attention exponent applied
    edge-major (PE transpose of the [4, S] logits) so clip/exp run on
    full 128 partitions; fused normalization (single pass) -> kg;
    G = (kg^2) @ (weight^2).T per tile ([ent, 32]) -> g_my.
  - AllGather g_my (bf16 [12544, 32] per core) -> g_full, viewed as a
    [n_pad/4, 128] table (4 G-rows per 256B row).
  - Stage B: per-group batched dma_gather of G4 rows (idx = tail//4 fits
    int16), (tail%4, type) selected via a host-precomputed one-hot;
    head-side G via one-hot matmul; unstable scatter-softmax (exact
    here: max w ~ 8e-4); fused [out | s] accumulation reusing the
    resident tail embeddings; normalize and store.
"""
import sys

for _p in ("/opt/trn_rl_repo", "/root/.axon_site/_ro/trn_rl_repo"):
    if _p not in sys.path:
        sys.path.insert(0, _p)

import numpy as np
import ml_dtypes

import concourse.bass as bass
import concourse.bacc as bacc
import concourse.mybir as mybir
import concourse.tile as tile
from concourse.bass import IndirectOffsetOnAxis
from concourse.bass_utils import run_bass_kernel_spmd
from concourse.masks import make_identity

FP = mybir.dt.float32
BF = mybir.dt.bfloat16
I32 = mybir.dt.int32
I16 = mybir.dt.int16

# Problem constants
N_ENT = 100000
D = 128
H = 4
DH = 32
R = 32
NCORE = 8
TILE = 128
TPC = 98            # tiles per core
N_PER = TPC * TILE  # 12544
N_PAD = N_PER * NCORE  # 100352
CPT = 5             # chunks (128 edge slots) per tile; max tile load is 576
GT = 7              # tiles per stage-B gather group
NGRP = TPC // GT    # 14
S = CPT * 128       # 640 edge slots per tile
GI = S * GT // 16   # gidx cols per group (280)


def _ap_append(ap, dims):
    """AP with extra broadcast/stride dims appended."""
    return bass.AP(tensor=ap.tensor, offset=ap.offset,
                   ap=[list(p) for p in ap.ap] + [list(d) for d in dims])


def _ap_insert(ap, pos, dims):
    base = [list(p) for p in ap.ap]
    return bass.AP(tensor=ap.tensor, offset=ap.offset,
                   ap=base[:pos] + [list(d) for d in dims] + base[pos:])


def _bcast(src_ap, parts):
    """Partition-broadcast a [1, S] DRAM AP to [parts, S] for DMA."""
    return bass.AP(tensor=src_ap.tensor, offset=src_ap.offset,
                   ap=[[0, parts]] + [list(p) for p in src_ap.ap[1:]])


def build(ncore=NCORE, tpc=TPC, cpt=CPT, with_cc=True):
    n_per = tpc * TILE
    n_pad = n_per * ncore
    nch = tpc * cpt

    nc = bacc.Bacc()
    ent = nc.dram_tensor("ent", [n_pad, D], BF, kind="ExternalInput")
    myrowsT = nc.dram_tensor("myrowsT", [n_per, D], BF, kind="ExternalInput")
    tailidx = nc.dram_tensor("tailidx", [128, nch], I32, kind="ExternalInput")
    hloc = nc.dram_tensor("hloc", [128, nch], BF, kind="ExternalInput")
    typ = nc.dram_tensor("typ", [128, nch], BF, kind="ExternalInput")
    hrow = nc.dram_tensor("hrow", [tpc, S], BF, kind="ExternalInput")
    trow = nc.dram_tensor("trow", [tpc, S], BF, kind="ExternalInput")
    qT = nc.dram_tensor("qT", [D, D], BF, kind="ExternalInput")
    kT = nc.dram_tensor("kT", [D, D], BF, kind="ExternalInput")
    vT = nc.dram_tensor("vT", [D, D], BF, kind="ExternalInput")
    wgt = nc.dram_tensor("wgt", [R, D], BF, kind="ExternalInput")
    w2T = nc.dram_tensor("w2T", [D, R], BF, kind="ExternalInput")
    hmask = nc.dram_tensor("hmask", [D, H], BF, kind="ExternalInput")
    iotac_in = nc.dram_tensor("iotac", [128, S], BF, kind="ExternalInput")
    iotar_in = nc.dram_tensor("iotar", [128, S], BF, kind="ExternalInput")
    iotar32_in = nc.dram_tensor("iotar32", [128, CPT * R], BF,
                                kind="ExternalInput")
    out_d = nc.dram_tensor("out", [n_per, D], FP, kind="ExternalOutput")

    g_my = nc.dram_tensor("g_my", [n_per, R], BF)
    if ncore > 4:
        g_full = nc.dram_tensor("g_full", [n_pad, R], BF, addr_space="Shared")
    else:
        g_full = nc.dram_tensor("g_full", [n_pad, R], BF)

    def mmN(out_fn, lhsT, rhs_fn, n_total, **kw):
        for off in range(0, n_total, 512):
            n = min(512, n_total - off)
            nc.tensor.matmul(out=out_fn(off, n), lhsT=lhsT, rhs=rhs_fn(off, n),
                             start=True, stop=True, **kw)

    with tile.TileContext(nc) as tc:
        with (
            tc.tile_pool(name="consts", bufs=1) as consts,
            tc.tile_pool(name="tep", bufs=1) as tep,
            tc.tile_pool(name="asb", bufs=2) as asb,
            tc.tile_pool(name="psA", bufs=2, space="PSUM") as psA,
            tc.tile_pool(name="psB", bufs=2, space="PSUM") as psB,
            tc.tile_pool(name="acc", bufs=1, space="PSUM") as accp,
            tc.tile_pool(name="tsb", bufs=2) as tsb,
            tc.tile_pool(name="cep", bufs=2) as cep,
        ):
            # ---------- constants ----------
            ident = consts.tile([128, 128], BF, tag="ident")
            make_identity(nc, ident[:])
            qT_s = consts.tile([D, D], BF, tag="qT")
            kT_s = consts.tile([D, D], BF, tag="kT")
            vT_s = consts.tile([D, D], BF, tag="vT")
            wgt_s = consts.tile([R, D], BF, tag="wgt")
            w2T_s = consts.tile([D, R], BF, tag="w2T")
            hm_s = consts.tile([D, H], BF, tag="hm")
            iotac_s = consts.tile([128, S], BF, tag="iotac")
            iotar_s = consts.tile([128, S], BF, tag="iotar")
            iotar32_s = consts.tile([128, CPT * R], BF, tag="iotar32")
            for dst, src in ((qT_s, qT), (kT_s, kT), (vT_s, vT), (wgt_s, wgt),
                             (w2T_s, w2T), (hm_s, hmask),
                             (iotac_s, iotac_in), (iotar_s, iotar_in),
                             (iotar32_s, iotar32_in)):
                nc.sync.dma_start(out=dst[:], in_=src[:])
            tidx_s = consts.tile([128, nch], I32, tag="tidx")
            hloc_s = consts.tile([128, nch], BF, tag="hloc")
            typ_s = consts.tile([128, nch], BF, tag="typ")
            nc.sync.dma_start(out=tidx_s[:], in_=tailidx[:])
            nc.sync.dma_start(out=hloc_s[:], in_=hloc[:])
            nc.sync.dma_start(out=typ_s[:], in_=typ[:])

            # resident gathered-tail tiles, one per tile of entities
            te_tiles = [tep.tile([128, cpt, D + 1], BF, tag=f"Te{t}",
                                 name=f"Te{t}")
                        for t in range(tpc)]

            def gather_tile(t):
                Te = te_tiles[t]
                for k in range(cpt):
                    nc.gpsimd.indirect_dma_start(
                        out=Te[:, k, 0:D], out_offset=None, in_=ent[:],
                        in_offset=IndirectOffsetOnAxis(
                            ap=tidx_s[:, t * cpt + k:t * cpt + k + 1], axis=0),
                    )
                nc.vector.memset(Te[:, :, D:D + 1], 1.0)

            PREF = 3
            for t in range(min(PREF, tpc)):
                gather_tile(t)

            # ---------- stage A ----------
            for t in range(tpc):
                if t + PREF < tpc:
                    gather_tile(t + PREF)
                Te = te_tiles[t]
                j0 = t * cpt

                E_T = tsb.tile([128, D], BF, tag="E_T")
                nc.sync.dma_start(out=E_T[:], in_=myrowsT[t * 128:(t + 1) * 128, :])
                q_ps = psB.tile([128, 512], FP, tag="B")
                nc.tensor.matmul(out=q_ps[:, 0:128], lhsT=E_T[:], rhs=qT_s[:],
                                 start=True, stop=True)
                Q_s = tsb.tile([128, D], BF, tag="Q_s")
                nc.scalar.activation(out=Q_s[:], in_=q_ps[:, 0:128],
                                     func=mybir.ActivationFunctionType.Copy)

                hbc = asb.tile([128, S], BF, tag="hbc")
                nc.sync.dma_start(out=hbc[:], in_=_bcast(hrow[t:t + 1, :], 128))
                tbc = asb.tile([R, S], BF, tag="tbc")
                nc.sync.dma_start(out=tbc[:], in_=_bcast(trow[t:t + 1, :], R))
                oh_entT = asb.tile([128, S], BF, tag="oh_entT")
                nc.vector.tensor_tensor(out=oh_entT[:], in0=hbc[:], in1=iotac_s[:],
                                        op=mybir.AluOpType.is_equal)
                oR_T = asb.tile([R, S], BF, tag="oR_T")
                nc.vector.tensor_tensor(out=oR_T[:], in0=tbc[:],
                                        in1=iotac_s[0:R, :],
                                        op=mybir.AluOpType.is_equal)
                oh_e = asb.tile([128, cpt, 128], BF, tag="oh_e")
                nc.vector.tensor_tensor(
                    out=oh_e[:],
                    in0=_ap_append(hloc_s[:, j0:j0 + cpt], [[0, 128]]),
                    in1=iotar_s[:].rearrange("p (c e) -> p c e", c=cpt),
                    op=mybir.AluOpType.is_equal)

                # T_T via PE transposes
                tt_ps = psA.tile([128, S], BF, tag="A")
                for k in range(cpt):
                    nc.tensor.transpose(out=tt_ps[:, k * 128:(k + 1) * 128],
                                        in_=Te[:, k, 0:D], identity=ident[:])
                T_T = asb.tile([128, S], BF, tag="T_T")
                nc.scalar.activation(out=T_T[:], in_=tt_ps[:],
                                     func=mybir.ActivationFunctionType.Copy)

                # rel_T = wgt.T @ oR_T ; neigh_T = T_T * rel_T
                rel_ps = psA.tile([128, S], FP, tag="A")
                mmN(lambda o, n: rel_ps[:, o:o + n], wgt_s[:],
                    lambda o, n: oR_T[:, o:o + n], S)
                neigh = asb.tile([128, S], BF, tag="neigh")
                nc.vector.tensor_mul(out=neigh[:], in0=T_T[:], in1=rel_ps[:])
                # k_T = kT.T @ neigh
                k_ps = psA.tile([128, S], FP, tag="A")
                mmN(lambda o, n: k_ps[:, o:o + n], kT_s[:],
                    lambda o, n: neigh[:, o:o + n], S)
                kTs = asb.tile([128, S], BF, tag="kTs")
                nc.scalar.activation(out=kTs[:], in_=k_ps[:],
                                     func=mybir.ActivationFunctionType.Copy)
                # q_edge_T = Q.T @ oh_entT
                qe_ps = psA.tile([128, S], FP, tag="A")
                mmN(lambda o, n: qe_ps[:, o:o + n], Q_s[:],
                    lambda o, n: oh_entT[:, o:o + n], S)
                qk = asb.tile([128, S], BF, tag="qk")
                nc.vector.tensor_mul(out=qk[:], in0=kTs[:], in1=qe_ps[:])
                # att = hmask.T @ qk  [H, S] feature-major
                att_ps = psA.tile([128, S], FP, tag="A")
                mmN(lambda o, n: att_ps[0:H, o:o + n], hm_s[:],
                    lambda o, n: qk[:, o:o + n], S)
                att_sb = asb.tile([H, S], BF, tag="att_sb")
                nc.vector.tensor_copy(out=att_sb[:], in_=att_ps[0:H, 0:S])
                # edge-major logits [128, cpt, H] via PE transposes
                ae_ps = psB.tile([128, 512], BF, tag="B")
                for k in range(cpt):
                    nc.tensor.transpose(out=ae_ps[:, k * H:(k + 1) * H],
                                        in_=att_sb[:, k * 128:(k + 1) * 128],
                                        identity=ident[0:H, 0:H])
                attc = asb.tile([128, cpt * H], FP, tag="attc")
                nc.vector.tensor_scalar_min(out=attc[:], in0=ae_ps[:, 0:cpt * H],
                                            scalar1=10.0)
                nc.vector.tensor_scalar_max(out=attc[:], in0=attc[:],
                                            scalar1=-10.0)
                expE = asb.tile([128, cpt, H], BF, tag="expE")
                nc.scalar.activation(
                    out=expE[:].rearrange("p c h -> p (c h)"), in_=attc[:],
                    func=mybir.ActivationFunctionType.Exp)

                # v edge-major
                v_ps = psA.tile([128, S], FP, tag="A")
                for k in range(cpt):
                    nc.tensor.matmul(out=v_ps[:, k * 128:(k + 1) * 128],
                                     lhsT=neigh[:, k * 128:(k + 1) * 128],
                                     rhs=vT_s[:], start=True, stop=True)
                vx = asb.tile([128, cpt, 132], BF, tag="vx")
                for h in range(H):
                    eh = expE[:, :, h:h + 1]
                    nc.vector.tensor_mul(
                        out=vx[:, :, h * DH:(h + 1) * DH],
                        in0=bass.AP(tensor=v_ps[:].tensor,
                                    offset=v_ps[:].offset + h * DH,
                                    ap=[list(v_ps[:].ap[0]), [128, cpt],
                                        [1, DH]]),
                        in1=bass.AP(tensor=eh.tensor, offset=eh.offset,
                                    ap=[list(eh.ap[0]), list(eh.ap[1]),
                                        [0, DH]]))
                nc.vector.tensor_copy(out=vx[:, :, 128:132], in_=expE[:])

                kgu = accp.tile([128, 132], FP, tag="kgu")
                for k in range(cpt):
                    nc.tensor.matmul(out=kgu[:, 0:132], lhsT=oh_e[:, k, :],
                                     rhs=vx[:, k, :],
                                     start=(k == 0), stop=(k == cpt - 1))

                # tile epilogue: kg, G
                rnorm = tsb.tile([128, H], FP, tag="rnorm")
                nc.vector.tensor_scalar_add(out=rnorm[:], in0=kgu[:, 128:132],
                                            scalar1=1e-8)
                nc.vector.reciprocal(out=rnorm[:], in_=rnorm[:])
                kg_sb = tsb.tile([128, D], BF, tag="kg_sb")
                nc.vector.tensor_mul(
                    out=kg_sb[:].rearrange("p (h e) -> p h e", h=H),
                    in0=kgu[:, 0:128].rearrange("p (h e) -> p h e", h=H),
                    in1=_ap_append(rnorm[:], [[0, DH]]))
                gp = psB.tile([128, 512], BF, tag="B")
                nc.tensor.transpose(out=gp[:, 0:128], in_=kg_sb[:],
                                    identity=ident[:])
                kg2T = tsb.tile([128, 128], BF, tag="kg2T")
                nc.scalar.square(out=kg2T[:], in_=gp[:, 0:128])
                gf = psB.tile([128, 512], FP, tag="B")
                nc.tensor.matmul(out=gf[:, 0:R], lhsT=kg2T[:], rhs=w2T_s[:],
                                 start=True, stop=True)
                g_sb = tsb.tile([128, R], BF, tag="g_sb")
                nc.vector.tensor_copy(out=g_sb[:], in_=gf[:, 0:R])
                nc.sync.dma_start(out=g_my[t * 128:(t + 1) * 128, :], in_=g_sb[:])

            # ---------- AllGather G ----------
            if with_cc:
                nc.gpsimd.collective_compute(
                    "AllGather", mybir.AluOpType.bypass,
                    replica_groups=[list(range(ncore))],
                    ins=[g_my[:, :]], outs=[g_full[:, :]],
                )
            else:
                nc.sync.dma_start(out=g_full[0:n_per, :], in_=g_my[:, :])

            # ---------- stage B ----------
            gt_tiles = [cep.tile([128, cpt, R], BF, tag=f"Gt{t % 6}",
                                 name=f"Gt{t}")
                        for t in range(6)]

            def gather_g(t):
                Gt = gt_tiles[t % 6]
                for k in range(cpt):
                    nc.gpsimd.indirect_dma_start(
                        out=Gt[:, k, :], out_offset=None, in_=g_full[:],
                        in_offset=IndirectOffsetOnAxis(
                            ap=tidx_s[:, t * cpt + k:t * cpt + k + 1], axis=0),
                    )
                return Gt

            PREFB = 3
            for t in range(min(PREFB, tpc)):
                gather_g(t)

            for t in range(tpc):
                if t + PREFB < tpc:
                    gather_g(t + PREFB)
                Gt = gt_tiles[t % 6]
                Te = te_tiles[t]
                j0 = t * cpt

                hbc = asb.tile([128, S], BF, tag="hbc")
                nc.sync.dma_start(out=hbc[:], in_=_bcast(hrow[t:t + 1, :], 128))
                oh_entT = asb.tile([128, S], BF, tag="oh_entT")
                nc.vector.tensor_tensor(out=oh_entT[:], in0=hbc[:], in1=iotac_s[:],
                                        op=mybir.AluOpType.is_equal)
                oh_e = asb.tile([128, cpt, 128], BF, tag="oh_e")
                nc.vector.tensor_tensor(
                    out=oh_e[:],
                    in0=_ap_append(hloc_s[:, j0:j0 + cpt], [[0, 128]]),
                    in1=iotar_s[:].rearrange("p (c e) -> p c e", c=cpt),
                    op=mybir.AluOpType.is_equal)
                oR_e = asb.tile([128, cpt, R], BF, tag="oR_e")
                nc.vector.tensor_tensor(
                    out=oR_e[:],
                    in0=_ap_append(typ_s[:, j0:j0 + cpt], [[0, R]]),
                    in1=iotar32_s[:].rearrange("p (c r) -> p c r", c=cpt),
                    op=mybir.AluOpType.is_equal)

                G_tile = tsb.tile([128, R], BF, tag="G_tile")
                nc.sync.dma_start(out=G_tile[:],
                                  in_=g_my[t * 128:(t + 1) * 128, :])
                gh_ps = psA.tile([128, S], FP, tag="A")
                for k in range(cpt):
                    nc.tensor.matmul(out=gh_ps[:, k * R:(k + 1) * R],
                                     lhsT=oh_entT[:, k * 128:(k + 1) * 128],
                                     rhs=G_tile[:], start=True, stop=True)
                scrh = asb.tile([128, cpt * R], FP, tag="scrh")
                nc.vector.tensor_mul(
                    out=scrh[:].rearrange("p (c r) -> p c r", c=cpt),
                    in0=gh_ps[:, 0:cpt * R].rearrange("p (c r) -> p c r", c=cpt),
                    in1=oR_e[:])
                hr2 = asb.tile([128, cpt], FP, tag="hr2")
                nc.vector.tensor_reduce(
                    out=hr2[:], in_=scrh[:].rearrange("p (c r) -> p c r", c=cpt),
                    axis=mybir.AxisListType.X, op=mybir.AluOpType.add)

                scrt = asb.tile([128, cpt, R], FP, tag="scrt")
                nc.vector.tensor_mul(out=scrt[:], in0=Gt[:], in1=oR_e[:])
                tr2 = asb.tile([128, cpt], FP, tag="tr2")
                nc.vector.tensor_reduce(
                    out=tr2[:], in_=scrt[:],
                    axis=mybir.AxisListType.X, op=mybir.AluOpType.add)

                expw = asb.tile([128, cpt], FP, tag="expw")
                nc.vector.tensor_mul(out=expw[:], in0=hr2[:], in1=tr2[:])
                expwb = asb.tile([128, cpt], BF, tag="expwb")
                nc.scalar.activation(out=expwb[:], in_=expw[:],
                                     func=mybir.ActivationFunctionType.Exp)
                mske = asb.tile([128, cpt, 128], BF, tag="mske")
                nc.vector.tensor_mul(
                    out=mske[:], in0=oh_e[:],
                    in1=_ap_append(expwb[:], [[0, 128]]))

                sout = accp.tile([128, 132], FP, tag="kgu")
                for k in range(cpt):
                    nc.tensor.matmul(out=sout[:, 0:129], lhsT=mske[:, k, :],
                                     rhs=Te[:, k, :],
                                     start=(k == 0), stop=(k == cpt - 1))

                rs = tsb.tile([128, 1], FP, tag="rs")
                nc.vector.tensor_scalar_add(out=rs[:], in0=sout[:, 128:129],
                                            scalar1=1e-30)
                nc.vector.reciprocal(out=rs[:], in_=rs[:])
                o_sb = tsb.tile([128, D], FP, tag="o_sb")
                nc.vector.tensor_scalar_mul(out=o_sb[:], in0=sout[:, 0:128],
                                            scalar1=rs[:])
                nc.sync.dma_start(out=out_d[t * 128:(t + 1) * 128, :], in_=o_sb[:])

    nc.finalize()
    return nc


def host_prep(entity_emb, weight, qTrans, kTrans, vTrans, edge_index, edge_type,
              ncore=NCORE, tpc=TPC, cpt=CPT):
    """Sort/shard/pad edges; build all per-core input dicts."""
    n_per = tpc * TILE
    nch = tpc * cpt
    slots = cpt * 128

    head = np.asarray(edge_index[0], dtype=np.int64)
    tail = np.asarray(edge_index[1], dtype=np.int64)
    etype = np.asarray(edge_type, dtype=np.int64) - 1

    order = np.argsort(head, kind="stable")
    hs, ts, rs = head[order], tail[order], etype[order]
    tile_of = hs // TILE
    n_tiles = ncore * tpc
    counts = np.bincount(tile_of, minlength=n_tiles)
    assert counts.max() <= slots, f"tile overflow: {counts.max()} > {slots}"
    tstart = np.concatenate([[0], np.cumsum(counts)])

    tails_sl = np.zeros((ncore, tpc, slots), dtype=np.int64)
    hloc_sl = np.full((ncore, tpc, slots), 255, dtype=np.float32)
    type_sl = np.full((ncore, tpc, slots), R, dtype=np.float32)
    for g in range(n_tiles):
        c, t = g // tpc, g % tpc
        n = counts[g]
        sl = slice(tstart[g], tstart[g] + n)
        tails_sl[c, t, :n] = ts[sl]
        hloc_sl[c, t, :n] = hs[sl] - g * TILE
        type_sl[c, t, :n] = rs[sl]

    def to_dev(a, dt):
        return np.ascontiguousarray(
            a.reshape(ncore, nch, 128).transpose(0, 2, 1)).astype(dt)

    tails_d = to_dev(tails_sl, np.int32)
    hloc_d = to_dev(hloc_sl, ml_dtypes.bfloat16)
    type_d = to_dev(type_sl, ml_dtypes.bfloat16)

    n_pad_rows = ncore * n_per
    ent_raw = np.asarray(entity_emb, dtype=np.float32)
    ent = np.zeros((n_pad_rows, D), np.float32)
    ent[:ent_raw.shape[0]] = ent_raw
    ent_bf = ent.astype(ml_dtypes.bfloat16)

    wgt = np.asarray(weight, dtype=np.float32)
    w2T = np.ascontiguousarray((wgt ** 2).T)
    hmask = np.zeros((D, H), np.float32)
    for h in range(H):
        hmask[h * DH:(h + 1) * DH, h] = 1.0
    iotac = np.tile(np.arange(128, dtype=np.float32)[:, None], (1, slots))
    iotar = np.tile(np.arange(128, dtype=np.float32), (128, cpt))
    iotar32 = np.tile(np.arange(R, dtype=np.float32), (128, cpt))

    shared = {
        "ent": ent_bf,
        "qT": np.asarray(qTrans, np.float32).astype(ml_dtypes.bfloat16),
        "kT": np.asarray(kTrans, np.float32).astype(ml_dtypes.bfloat16),
        "vT": np.asarray(vTrans, np.float32).astype(ml_dtypes.bfloat16),
        "wgt": wgt.astype(ml_dtypes.bfloat16),
        "w2T": w2T.astype(ml_dtypes.bfloat16),
        "hmask": hmask.astype(ml_dtypes.bfloat16),
        "iotac": iotac.astype(ml_dtypes.bfloat16),
        "iotar": iotar.astype(ml_dtypes.bfloat16),
        "iotar32": iotar32.astype(ml_dtypes.bfloat16),
    }
    in_maps = []
    for c in range(ncore):
        rows = ent[c * n_per:(c + 1) * n_per]
        myT = rows.reshape(tpc, TILE, D).transpose(0, 2, 1).reshape(n_per, D)
        in_maps.append(dict(
            shared,
            myrowsT=np.ascontiguousarray(myT).astype(ml_dtypes.bfloat16),
            tailidx=tails_d[c],
            hloc=hloc_d[c], typ=type_d[c],
            hrow=np.ascontiguousarray(hloc_sl[c]).astype(ml_dtypes.bfloat16),
            trow=np.ascontiguousarray(type_sl[c]).astype(ml_dtypes.bfloat16),
        ))
    return in_maps


_NC_CACHE = {}


def kernel(entity_emb, user_emb, interact_mat, weight, qTrans, kTrans, vTrans,
           edge_index, edge_type, layer=0):
    key = "full"
    if key not in _NC_CACHE:
        _NC_CACHE[key] = build()
    nc = _NC_CACHE[key]
    in_maps = host_prep(entity_emb, weight, qTrans, kTrans, vTrans,
                        edge_index, edge_type)
    res = run_bass_kernel_spmd(nc, in_maps, list(range(NCORE)))
    out = np.concatenate([res.results[c]["out"] for c in range(NCORE)], axis=0)
    return np.ascontiguousarray(out[:N_ENT]).astype(np.float32)
